# revision 13
# baseline (speedup 1.0000x reference)
"""DataAwareGCN (2-layer GCN, 100k nodes / 1.6M edges) on 8 TRN2 NeuronCores.

Strategy (graph/data parallel, dst-sharded):
  - Nodes are partitioned into 8 contiguous blocks of 12500 (padded to 12544 = 98*128).
  - Normalization folds into per-row scales:  out = relu(dinv * (S @ (dinv*h)) + b)
    where S is the 0/1 adjacency (incl. self loop) and h = x @ W.  No per-edge coefs.
  - Each core: transform its x-block (PE matmul), scale rows by dinv, AllGather into a
    full feature table in DRAM, then aggregate its destination tiles with dma_gather
    (per-edge source-row gather) + one-hot selection matmuls that segment-sum into PSUM.
  - Edge buckets: (dst_tile (128 dsts), src-row window (32768 rows, int16 gather idx)),
    padded to multiples of 128 and to uniform sizes across cores so one SPMD program
    serves all 8 cores.  Index/one-hot metadata is shared by both layers.
"""

import sys
import os

for _p in ("/opt/trn_rl_repo", os.path.expanduser("~/.axon_site/_ro/trn_rl_repo")):
    if os.path.isdir(_p) and _p not in sys.path:
        sys.path.insert(0, _p)
        break

import numpy as np

N_CORES = 8
N_NODES = 100000
NPC = 12500          # nodes per core (real)
T = 98               # dst tiles per core
NPAD = T * 128       # 12544, padded nodes per core
ROWS = NPAD * N_CORES  # 100352 table rows
WIN = 32768          # gather index window (int16)
NW = (ROWS + WIN - 1) // WIN  # 4
F0, F1, F2 = 256, 128, 64
P = 128
G = 4                # dst tiles per gather group

_CACHE = {}


# ---------------------------------------------------------------- host prep

def _prep_graph(edge_index: np.ndarray):
    """Bucket edges by (core, dst_tile, src_window); uniform bucket sizes across
    cores (SPMD).  Returns per-core device arrays + a static schedule."""
    src = edge_index[0].astype(np.int64)
    dst = edge_index[1].astype(np.int64)
    deg = np.bincount(dst, minlength=N_NODES).astype(np.float32) + 1.0

    per_core = []
    cnts = np.zeros((N_CORES, T * NW), np.int64)
    for c in range(N_CORES):
        m = (dst >= c * NPC) & (dst < (c + 1) * NPC)
        es = np.concatenate([src[m], np.arange(c * NPC, (c + 1) * NPC)])
        ed = np.concatenate([dst[m] - c * NPC, np.arange(NPC)])
        erow = (es // NPC) * NPAD + (es % NPC)   # table row of source
        t = ed >> 7
        w = erow // WIN
        key = t * NW + w
        order = np.argsort(key, kind="stable")
        erow_s = erow[order]
        dloc_s = (ed - t * 128)[order]           # 0..127 within tile
        cnt = np.bincount(key, minlength=T * NW)
        cnts[c] = cnt
        off = np.zeros(T * NW + 1, np.int64)
        np.cumsum(cnt, out=off[1:])
        per_core.append((erow_s, dloc_s, off))

    Bmax = cnts.max(axis=0)
    Bp = ((Bmax + 127) // 128) * 128             # padded bucket size (0 stays 0)

    # assembly order: group-major, then window, then tile within group
    groups = [list(range(g0, min(g0 + G, T))) for g0 in range(0, T, G)]
    sched = []            # [ per group: [ per window: dict(cnt, i0, chunks) ] ]
    bucket_dst = {}       # (t, w) -> dest offset in assembled arrays
    pos = 0
    nchunks = 0
    tile_total = np.zeros(T, np.int64)   # total chunks per tile
    for ts in groups:
        for t in ts:
            for w in range(NW):
                tile_total[t] += Bp[t * NW + w] // 128
    tile_seen = np.zeros(T, np.int64)
    for ts in groups:
        gw = []
        for w in range(NW):
            cnt = int(sum(Bp[t * NW + w] for t in ts))
            chunks = []
            i0 = pos
            for t in ts:
                b = int(Bp[t * NW + w])
                if b == 0:
                    continue
                bucket_dst[(t, w)] = pos
                for _ in range(b // 128):
                    tile_seen[t] += 1
                    chunks.append((t, nchunks,
                                   tile_seen[t] == 1,
                                   tile_seen[t] == tile_total[t]))
                    nchunks += 1
                    pos += 128
            gw.append({"cnt": cnt, "i0": i0, "chunks": chunks})
        sched.append({"tiles": ts, "w": gw})
    nidx = pos

    # per-core assembled arrays
    idx16_all, dstloc_all, deg_all = [], [], []
    for c in range(N_CORES):
        erow_s, dloc_s, off = per_core[c]
        idxoff = np.zeros(nidx, np.int32)
        dloc = np.full(nidx, 300.0, np.float32)
        for (t, w), d0 in bucket_dst.items():
            k = t * NW + w
            n = int(cnts[c, k])
            if n:
                sl = slice(off[k], off[k] + n)
                idxoff[d0:d0 + n] = erow_s[sl] - w * WIN
                dloc[d0:d0 + n] = dloc_s[sl]
        w16 = idxoff.reshape(nidx // 16, 16).T.astype(np.int16)   # [16, nidx/16]
        idx16_all.append(np.ascontiguousarray(np.tile(w16, (8, 1))))
        dstloc_all.append(np.ascontiguousarray(dloc.reshape(nchunks, 128).T))
        dg = np.ones(NPAD, np.float32)
        dg[:NPC] = deg[c * NPC:(c + 1) * NPC]
        deg_all.append(np.ascontiguousarray(dg.reshape(T, 128).T))  # [128, T]

    return dict(sched=sched, nidx=nidx, nchunks=nchunks,
                idx16=idx16_all, dstloc=dstloc_all, deg=deg_all)


# ---------------------------------------------------------------- device build

def _build(meta):
    import concourse.bacc as bacc
    import concourse.mybir as mybir
    import concourse.tile as tile
    from concourse.masks import make_identity

    phase = os.environ.get("GCN_PHASE", "full")  # tf | ag | agg1 | full
    sched, nidx, nchunks = meta["sched"], meta["nidx"], meta["nchunks"]
    f32, i16 = mybir.dt.float32, mybir.dt.int16
    AOT = mybir.AluOpType

    nc = bacc.Bacc(num_devices=N_CORES)
    xT = nc.declare_dram_parameter("xT", [F0, NPAD], f32, isOutput=False)
    W1 = nc.declare_dram_parameter("W1", [F0, F1], f32, isOutput=False)
    W2 = nc.declare_dram_parameter("W2", [F1, F2], f32, isOutput=False)
    B1 = nc.declare_dram_parameter("B1", [P, F1], f32, isOutput=False)
    B2 = nc.declare_dram_parameter("B2", [P, F2], f32, isOutput=False)
    DEG = nc.declare_dram_parameter("DEG", [P, T], f32, isOutput=False)
    IDX = nc.declare_dram_parameter("IDX", [P, nidx // 16], i16, isOutput=False)
    DLOC = nc.declare_dram_parameter("DLOC", [P, nchunks], f32, isOutput=False)
    OUT = nc.declare_dram_parameter("OUT", [NPAD, F2], f32, isOutput=True)

    inb1 = nc.dram_tensor("inb1", [NPAD, F1], f32)
    tab1 = nc.dram_tensor("tab1", [ROWS, F1], f32, addr_space="Shared")
    inb2 = nc.dram_tensor("inb2", [NPAD, F2], f32)
    tab2 = nc.dram_tensor("tab2", [ROWS, F2], f32, addr_space="Shared")

    # max gather bucket size (in chunks) for gbuf slot sizing
    max_gw = max(gw["cnt"] for grp in sched for gw in grp["w"]) // 128

    with tile.TileContext(nc) as tc:
        with (
            tc.tile_pool(name="const", bufs=1) as cpool,
            tc.tile_pool(name="xload", bufs=2) as xpool,
            tc.tile_pool(name="gbuf", bufs=4) as gpool,
            tc.tile_pool(name="work", bufs=3) as wpool,
            tc.tile_pool(name="hout", bufs=4) as hpool,
            tc.tile_pool(name="pagg", bufs=5, space="PSUM") as pagg,
            tc.tile_pool(name="ptf", bufs=2, space="PSUM") as ptf,
            tc.tile_pool(name="ptf2", bufs=1, space="PSUM") as ptf2,
        ):
            # ---------------- constants
            w1t = [cpool.tile([P, F1], f32, tag=f"w1_{k}", name=f"w1_{k}") for k in range(2)]
            for k in range(2):
                nc.sync.dma_start(out=w1t[k][:], in_=W1[k * 128:(k + 1) * 128, :])
            w2t = cpool.tile([P, F2], f32, tag="w2")
            nc.sync.dma_start(out=w2t[:], in_=W2[:, :])
            b1t = cpool.tile([P, F1], f32, tag="b1")
            nc.sync.dma_start(out=b1t[:], in_=B1[:, :])
            b2t = cpool.tile([P, F2], f32, tag="b2")
            nc.sync.dma_start(out=b2t[:], in_=B2[:, :])
            degt = cpool.tile([P, T], f32, tag="deg")
            nc.sync.dma_start(out=degt[:], in_=DEG[:, :])
            dinv = cpool.tile([P, T], f32, tag="dinv")
            nc.vector.reciprocal(dinv[:], degt[:])
            nc.scalar.activation(dinv[:], dinv[:], mybir.ActivationFunctionType.Sqrt)

            iota = cpool.tile([P, P], f32, tag="iota")
            nc.gpsimd.iota(iota[:], pattern=[[1, P]], base=0, channel_multiplier=0,
                           allow_small_or_imprecise_dtypes=True)
            ident = cpool.tile([P, P], f32, tag="ident")
            make_identity(nc, ident[:])

            idxt = cpool.tile([P, nidx // 16], i16, tag="idx")
            nc.sync.dma_start(out=idxt[:], in_=IDX[:, :])
            dloct = cpool.tile([P, nchunks], f32, tag="dloc")
            nc.sync.dma_start(out=dloct[:], in_=DLOC[:, :])

            # ---------------- layer-1 transform: h~1 = dinv * (x @ W1) -> inb1
            SW = 1568  # node columns per x strip (98*128/8)
            for s0 in range(0, NPAD, SW):
                xt = [xpool.tile([P, SW], f32, tag=f"x{k}", name=f"xt_{k}_{s0}") for k in range(2)]
                for k in range(2):
                    nc.sync.dma_start(out=xt[k][:], in_=xT[k * 128:(k + 1) * 128, s0:s0 + SW])
                for off in range(0, SW, 128):
                    t_idx = (s0 + off) // 128
                    ph = ptf.tile([P, F1], f32, tag="tf", space="PSUM")
                    for k in range(2):
                        nc.tensor.matmul(out=ph[:], lhsT=xt[k][:, off:off + 128],
                                         rhs=w1t[k][:], start=(k == 0), stop=(k == 1))
                    h1s = hpool.tile([P, F1], f32, tag="h1s")
                    nc.vector.tensor_scalar(out=h1s[:], in0=ph[:],
                                            scalar1=dinv[:, t_idx:t_idx + 1], scalar2=None,
                                            op0=AOT.mult)
                    nc.sync.dma_start(out=inb1[t_idx * 128:(t_idx + 1) * 128, :], in_=h1s[:])
                    if phase == "tf":
                        nc.sync.dma_start(out=OUT[t_idx * 128:(t_idx + 1) * 128, :],
                                          in_=h1s[:, :F2])

            if phase != "tf":
                nc.gpsimd.collective_compute(
                    "AllGather", AOT.bypass,
                    replica_groups=[list(range(N_CORES))],
                    ins=[inb1.ap().opt()], outs=[tab1.ap().opt()],
                )
            if phase == "ag":
                nc.gpsimd.dma_start(out=OUT[:, :], in_=tab1[0:NPAD, :F2])

            # ---------------- aggregation layers
            def aggregate(layer, table, F, bias_t, post):
                for grp in sched:
                    gbufs = []
                    for w in range(NW):
                        gw = grp["w"][w]
                        if gw["cnt"] == 0:
                            gbufs.append(None)
                            continue
                        nch = gw["cnt"] // 128
                        gb = gpool.tile([P, max_gw * F1], f32, tag="gb")
                        gb3 = gb[:, :nch * F].rearrange("p (c f) -> p c f", f=F)
                        nc.gpsimd.dma_gather(
                            out_ap=gb3,
                            in_ap=table[w * WIN: min((w + 1) * WIN, ROWS), :],
                            idxs_ap=idxt[:, gw["i0"] // 16: (gw["i0"] + gw["cnt"]) // 16],
                            num_idxs=gw["cnt"],
                            num_idxs_reg=gw["cnt"],
                            elem_size=F,
                            single_packet=False,
                        )
                        gbufs.append(gb3)
                    psums = {}
                    for w in range(NW):
                        gw = grp["w"][w]
                        for ci, (t, cg, first, last) in enumerate(gw["chunks"]):
                            if first:
                                psums[t] = pagg.tile([P, F], f32, tag="agg", space="PSUM",
                                                     name=f"agg{layer}_{t}")
                            s_t = wpool.tile([P, P], f32, tag="sel")
                            nc.vector.tensor_scalar(
                                out=s_t[:], in0=iota[:],
                                scalar1=dloct[:, cg:cg + 1], scalar2=None,
                                op0=AOT.is_equal)
                            nc.tensor.matmul(out=psums[t][:], lhsT=s_t[:],
                                             rhs=gbufs[w][:, ci, :],
                                             start=first, stop=last)
                            if last:
                                post(t, psums.pop(t))

            # L1 post: out1=relu(dinv*ps+B1); h~2 = dinv*(out1 @ W2) -> inb2
            def post1(t, ps):
                o1 = hpool.tile([P, F1], f32, tag="o1")
                nc.vector.tensor_scalar(out=o1[:], in0=ps[:],
                                        scalar1=dinv[:, t:t + 1], scalar2=None,
                                        op0=AOT.mult)
                nc.vector.tensor_tensor(out=o1[:], in0=o1[:], in1=b1t[:], op=AOT.add)
                nc.vector.tensor_scalar(out=o1[:], in0=o1[:], scalar1=0.0, scalar2=None,
                                        op0=AOT.max)
                if phase == "agg1":
                    nc.sync.dma_start(out=OUT[t * 128:(t + 1) * 128, :], in_=o1[:, :F2])
                    return
                pT = ptf.tile([P, P], f32, tag="tf", space="PSUM")
                nc.tensor.transpose(out=pT[:], in_=o1[:], identity=ident[:])
                o1T = hpool.tile([P, P], f32, tag="o1T")
                nc.vector.tensor_copy(o1T[:], pT[:])
                p2 = ptf2.tile([P, F2], f32, tag="tf2", space="PSUM")
                nc.tensor.matmul(out=p2[:], lhsT=o1T[:], rhs=w2t[:], start=True, stop=True)
                h2s = hpool.tile([P, F2], f32, tag="h2s")
                nc.vector.tensor_scalar(out=h2s[:], in0=p2[:],
                                        scalar1=dinv[:, t:t + 1], scalar2=None,
                                        op0=AOT.mult)
                nc.sync.dma_start(out=inb2[t * 128:(t + 1) * 128, :], in_=h2s[:])

            if phase in ("agg1", "full"):
                aggregate(1, tab1, F1, b1t, post1)

            if phase == "full":
                nc.gpsimd.collective_compute(
                    "AllGather", AOT.bypass,
                    replica_groups=[list(range(N_CORES))],
                    ins=[inb2.ap().opt()], outs=[tab2.ap().opt()],
                )

                # L2 post: out = relu(dinv*ps + B2) -> OUT
                def post2(t, ps):
                    o2 = hpool.tile([P, F2], f32, tag="o2")
                    nc.vector.tensor_scalar(out=o2[:], in0=ps[:],
                                            scalar1=dinv[:, t:t + 1], scalar2=None,
                                            op0=AOT.mult)
                    nc.vector.tensor_tensor(out=o2[:], in0=o2[:], in1=b2t[:], op=AOT.add)
                    nc.vector.tensor_scalar(out=o2[:], in0=o2[:], scalar1=0.0, scalar2=None,
                                            op0=AOT.max)
                    nc.sync.dma_start(out=OUT[t * 128:(t + 1) * 128, :], in_=o2[:])

                aggregate(2, tab2, F2, b2t, post2)

    nc.finalize()
    return nc


# ---------------------------------------------------------------- entry point

def _get_compiled(edge_index):
    key = edge_index.tobytes()[:64], edge_index.shape
    ent = _CACHE.get("k")
    if ent is not None and ent[0] == key:
        return ent[1], ent[2]
    meta = _prep_graph(edge_index)
    nc = _build(meta)
    _CACHE["k"] = (key, nc, meta)
    return nc, meta


def _in_maps(meta, x, W1, b1, W2, b2):
    xTs = []
    for c in range(N_CORES):
        xt = np.zeros((F0, NPAD), np.float32)
        xt[:, :NPC] = x[c * NPC:(c + 1) * NPC].T
        xTs.append(xt)
    B1 = np.ascontiguousarray(np.broadcast_to(b1, (P, F1))).astype(np.float32)
    B2 = np.ascontiguousarray(np.broadcast_to(b2, (P, F2))).astype(np.float32)
    return [
        {"xT": xTs[c], "W1": np.ascontiguousarray(W1, np.float32),
         "W2": np.ascontiguousarray(W2, np.float32), "B1": B1, "B2": B2,
         "DEG": meta["deg"][c], "IDX": meta["idx16"][c], "DLOC": meta["dstloc"][c]}
        for c in range(N_CORES)
    ]


def kernel(x, edge_index, W1, b1, W2, b2):
    from concourse.bass_utils import run_bass_kernel_spmd
    x = np.asarray(x, np.float32)
    edge_index = np.asarray(edge_index)
    nc, meta = _get_compiled(edge_index)
    in_maps = _in_maps(meta, x, np.asarray(W1), np.asarray(b1), np.asarray(W2), np.asarray(b2))
    res = run_bass_kernel_spmd(nc, in_maps, core_ids=list(range(N_CORES)))
    out = np.concatenate([res.results[c]["OUT"][:NPC] for c in range(N_CORES)], axis=0)
    return out.astype(np.float32)
